# revision 1
# baseline (speedup 1.0000x reference)
"""Gated multi-head attention on 8 NeuronCores.

Sharding (hardcoded): core c -> (batch b = c // 4, head-group g = c % 4).
Data-parallel over B=2, tensor-parallel over the 16 heads in groups of 4.
Each core computes its 4 heads' attention plus the corresponding slice of
the output projection; the host sums the 4 head-group partials per batch
(bf16 partials) and adds the output bias.

Per-core kernel (bf16 matmul inputs, fp32 PSUM):
  kT[256,2048] = (Wk_slice.T).T @ x_k.T    (bias folded into the DVE
  qT[256,2048] likewise                     PSUM->SBUF eviction; gate
                                            sigmoid/sqrt(D) into q scale)
  v[2048,256]  natural [pos, dim] layout, bf16, ones column per head
  attention in head PAIRS (2h, 2h+1) that live at partition offsets 0/64
  of chunk c=h//2, so their K=64 QK^T matmuls tile-pack into rows 0-63 /
  64-127 of the PE array and run concurrently:
    per (qb 1024-block, pair), per key-chunk kc (16):
      S^T[k128, 2*1024] PSUM (head pair side by side) via 4 matmuls
      P^T = exp(S^T) on ACT as ONE [128,2048] activation (amortizes the
      ~352-cycle ACT instruction overhead; no max-subtraction, logits ~ +-4)
    per ql (128 q rows), per head: AV via 4-way column-tiled matmuls
      (lhsT = P^T slices [128,32] -> tile_position (0,32j)) accumulating
      [32,65] PSUM slices over 16 kc; col 64 = softmax denominator
      DVE: recip + per-head scale -> A[q, 256] bf16
  y[q,1024] partial = PE-transpose of A (bf16) then matmul with Wo slice;
  y stored bf16, summed on host in f32.
"""

import math
from collections import deque
from contextlib import ExitStack

import numpy as np

import concourse.bass as bass
import concourse.tile as tile
from concourse import mybir
from concourse.bass_utils import run_bass_kernel_spmd
from concourse.masks import make_identity

B = 2
N = 2048
E = 1024
H = 16
D = 64
NCORES = 8
GROUPS = NCORES // B      # head-groups per batch
HG = H // GROUPS          # heads per core
DH = HG * D               # 256 head-dims per core
P = 128

F32 = mybir.dt.float32
BF16 = mybir.dt.bfloat16
AF = mybir.ActivationFunctionType
ALU = mybir.AluOpType

TRACE = False
LAST_RESULTS = None

KC = E // P            # 8 contraction chunks over the embed dim
MC = DH // P           # 2 partition chunks over this core's head dims
NB = N // 512          # 4 blocks of 512 positions
KB = N // P            # 16 key-position chunks
NPAIR = HG // 2        # head pairs per core
NQB = N // 1024        # query 1024-blocks


def _split_drain_waits(nc):
    """The installed walrus build accepts only ONE sync-wait per instruction
    (one NEURON_ISA_TPB_EVENTS slot), but Tile emits several on drains,
    matmuls, etc.  Hoist all but the last wait onto dedicated single-wait
    NOPs ahead of the instruction on the same engine (the lowering newer
    walrus performs itself)."""
    n = 0
    for fn in nc.m.functions:
        for bb in fn.blocks:
            insts = bb.instructions
            idx = 0
            while idx < len(insts):
                inst = insts[idx]
                si = inst.sync_info
                if si is not None and len(si.on_wait) > 1:
                    waits = list(si.on_wait)
                    nops = []
                    for w in waits[:-1]:
                        n += 1
                        nop = mybir.InstNoOp(
                            name=f"waitsplit-{n}",
                            engine=inst.engine,
                            sync_info=mybir.SyncInfo(on_wait=[w], on_update=[]),
                            bass_nofuse=True,
                        )
                        nc.register_instruction(nop)
                        nops.append(nop)
                    inst.sync_info = mybir.SyncInfo(
                        on_wait=[waits[-1]], on_update=list(si.on_update))
                    insts[idx:idx] = nops
                    idx += len(nops)
                idx += 1
    return n


def _build(reps=1):
    nc = bass.Bass()
    # inputs arrive partition-major ([128, chunk, free]) so each loads as a
    # single DMA with 32KB contiguous per-partition runs
    xqT = nc.dram_tensor("xqT", [P, KC, N], BF16, kind="ExternalInput")
    xkT = nc.dram_tensor("xkT", [P, KC, N], BF16, kind="ExternalInput")
    xvT = nc.dram_tensor("xvT", [P, KC, N], BF16, kind="ExternalInput")
    wqT = nc.dram_tensor("wqT", [P, KC, DH], BF16, kind="ExternalInput")
    wkT = nc.dram_tensor("wkT", [P, KC, DH], BF16, kind="ExternalInput")
    wvT = nc.dram_tensor("wvT", [P, KC, DH], BF16, kind="ExternalInput")
    woB = nc.dram_tensor("woB", [P, MC, E], BF16, kind="ExternalInput")
    qscale = nc.dram_tensor("qscale", [DH], F32, kind="ExternalInput")
    qbias = nc.dram_tensor("qbias", [DH], F32, kind="ExternalInput")
    kbias = nc.dram_tensor("kbias", [DH], F32, kind="ExternalInput")
    vbias = nc.dram_tensor("vbias", [DH], F32, kind="ExternalInput")
    y = nc.dram_tensor("y", [N, E], BF16, kind="ExternalOutput")

    with ExitStack() as ctx:
        tc = ctx.enter_context(tile.TileContext(nc))
        const = ctx.enter_context(tc.tile_pool(name="const", bufs=1))
        xpool = ctx.enter_context(tc.tile_pool(name="xpool", bufs=2))
        wpool = ctx.enter_context(tc.tile_pool(name="wpool", bufs=2))
        wqpool = ctx.enter_context(tc.tile_pool(name="wqpool", bufs=1))
        wopool = ctx.enter_context(tc.tile_pool(name="wopool", bufs=1))
        qkpool = ctx.enter_context(tc.tile_pool(name="qkpool", bufs=4))
        vpool = ctx.enter_context(tc.tile_pool(name="vpool", bufs=KB))
        ptpool = ctx.enter_context(tc.tile_pool(name="ptpool", bufs=36))
        apool = ctx.enter_context(tc.tile_pool(name="apool", bufs=16))
        atpool = ctx.enter_context(tc.tile_pool(name="atpool", bufs=2))
        ypool = ctx.enter_context(tc.tile_pool(name="ypool", bufs=2))
        spool = ctx.enter_context(tc.tile_pool(name="spool", bufs=8))
        # PSUM: stq 2x2 banks + pp 2x1 + av 4x260B -> fits the 8x2KB banks
        pstq = ctx.enter_context(tc.tile_pool(name="pstq", bufs=2,
                                              space="PSUM"))
        ppp = ctx.enter_context(tc.tile_pool(name="ppp", bufs=2,
                                             space="PSUM"))
        pav = ctx.enter_context(tc.tile_pool(name="pav", bufs=2,
                                             space="PSUM"))

        # ---- DMA: inputs on the SP queue, weights on the ACT queue ----
        qs_sb = const.tile([P, MC], F32, name="qs")
        nc.scalar.dma_start(out=qs_sb,
                            in_=qscale[:].rearrange("(c p) -> p c", p=P))
        qb_sb = const.tile([P, MC], F32, name="qb")
        nc.scalar.dma_start(out=qb_sb,
                            in_=qbias[:].rearrange("(c p) -> p c", p=P))
        kb_sb = const.tile([P, MC], F32, name="kb")
        nc.scalar.dma_start(out=kb_sb,
                            in_=kbias[:].rearrange("(c p) -> p c", p=P))
        vb_ap = vbias[:]
        vb_bc = const.tile([P, DH], F32, name="vb")
        nc.gpsimd.dma_start(out=vb_bc, in_=bass.AP(
            tensor=vb_ap.tensor, offset=vb_ap.offset, ap=[[0, P]] + vb_ap.ap))

        def load_x(x_dram, engine):
            # 4 pieces into one tile: consumers start before the tail lands
            t = xpool.tile([P, KC, N], BF16, name="xs")
            for i in range(4):
                engine.dma_start(out=t[:, 2 * i:2 * i + 2, :],
                                 in_=x_dram[:, 2 * i:2 * i + 2, :])
            return [t[:, kc, :] for kc in range(KC)]

        def load_w(w_dram, pool, tag, engine):
            t = pool.tile([P, KC, DH], BF16, name=tag)
            engine.dma_start(out=t, in_=w_dram[:, :, :])
            return [t[:, kc, :] for kc in range(KC)]

        ident = const.tile([P, P], BF16)
        make_identity(nc, ident)

        def emit_body():
            # critical path: xk then xq on the SP queue; xv/wv/wo behind them;
            # wk/wq/consts on the (otherwise idle) ACT queue
            xk = load_x(xkT, nc.sync)
            wk_c = load_w(wkT, wpool, "ws", nc.scalar)
            wq_c = load_w(wqT, wqpool, "wq", nc.scalar)
            xq = load_x(xqT, nc.sync)
            xv = load_x(xvT, nc.sync)
            wv_c = load_w(wvT, wpool, "ws", nc.sync)
            wo_t = wopool.tile([P, MC, E], BF16, name="wo")
            nc.sync.dma_start(out=wo_t, in_=woB[:, :, :])
            wo_sb = [wo_t[:, c, :] for c in range(MC)]

            # ---- projections (group-major; evictions on DVE) ----
            kT = [qkpool.tile([P, N], BF16, name="kt") for _ in range(MC)]
            qT = [qkpool.tile([P, N], BF16, name="qt") for _ in range(MC)]

            def proj_group(xs, w_c, out_sb, c, nb, scale_sb, bias_sb):
                pt = ppp.tile([P, 512], F32, name="pp")
                for kc in range(KC):
                    nc.tensor.matmul(
                        pt,
                        lhsT=w_c[kc][:, c * P:(c + 1) * P],
                        rhs=xs[kc][:, nb * 512:(nb + 1) * 512],
                        start=(kc == 0), stop=(kc == KC - 1))
                if scale_sb is None:
                    nc.vector.tensor_scalar_add(
                        out=out_sb[c][:, nb * 512:(nb + 1) * 512], in0=pt,
                        scalar1=bias_sb[:, c:c + 1])
                else:
                    nc.vector.tensor_scalar(
                        out=out_sb[c][:, nb * 512:(nb + 1) * 512], in0=pt,
                        scalar1=scale_sb[:, c:c + 1], scalar2=bias_sb[:, c:c + 1],
                        op0=ALU.mult, op1=ALU.add)

            v_sb = []

            def vproj_group(m):
                vt = vpool.tile([P, HG, D + 1], BF16, name="vt")
                nc.gpsimd.memset(vt[:, :, D:D + 1], 1.0)
                pv = ppp.tile([P, 512], F32, name="pp")[:, :DH]
                for kc in range(KC):
                    nc.tensor.matmul(
                        pv,
                        lhsT=xv[kc][:, m * P:(m + 1) * P],
                        rhs=wv_c[kc],
                        start=(kc == 0), stop=(kc == KC - 1))
                nc.vector.tensor_add(
                    out=vt[:, :, 0:D],
                    in0=pv.rearrange("p (h d) -> p h d", h=HG),
                    in1=vb_bc.rearrange("p (h d) -> p h d", h=HG))
                v_sb.append(vt)

            # K proj fully (it fits inside the xk->xq DMA window), then the
            # first Q proj group; the remaining Q groups are fills
            for c in range(MC):
                for nb in range(NB):
                    proj_group(xk, wk_c, kT, c, nb, None, kb_sb)
            proj_group(xq, wq_c, qT, 0, 0, qs_sb, qb_sb)

            kq_fills = deque()
            for c, nb in ((1, 0), (0, 1), (1, 1), (0, 2), (1, 2), (0, 3),
                          (1, 3)):
                kq_fills.append(lambda c=c, nb=nb: proj_group(
                    xq, wq_c, qT, c, nb, qs_sb, qb_sb))
            v_fills = deque(
                (lambda m=m: vproj_group(m)) for m in range(KB))
            fills = deque()     # AV + O-proj: the ptile-recycling path

            # ---- attention: units of (512-query block, head pair) ----
            units = [(qs, pr) for qs in range(N // 512) for pr in range(NPAIR)]
            pts_cur = {}
            a_tiles_all = {}

            def get_a_tiles(qb):
                if qb not in a_tiles_all:
                    a_tiles_all[qb] = [apool.tile([P, DH], BF16, name="acc")
                                       for _ in range(8)]
                return a_tiles_all[qb]

            def emit_se_chunk(u, kc):
                qs, pr = u
                get_a_tiles(qs // 2)
                stp = pstq.tile([P, 1024], F32, name="stq")
                q0 = qs * 512
                for t in range(2):          # head 2*pr + t at partition 64*t
                    off = t * D
                    nc.tensor.matmul(
                        stp[:, t * 512:(t + 1) * 512],
                        lhsT=kT[pr][off:off + D, kc * P:(kc + 1) * P],
                        rhs=qT[pr][off:off + D, q0:q0 + 512],
                        start=True, stop=True)
                ptile = ptpool.tile([P, 1024], BF16, name="pt")
                nc.scalar.activation(out=ptile, in_=stp, func=AF.Exp)
                pts_cur.setdefault(u, {})[kc] = ptile

            def emit_av_group(u, ql, t):
                qs, pr = u
                h = 2 * pr + t
                pts = pts_cur[u]
                # full-bank tile: PSUM start_tensor_calc zeroing is 2KB-granular
                av = pav.tile([P, 512], F32, name="av")[:, 0:D + 1]
                for kc in range(KB):
                    for j in range(4):
                        nc.tensor.matmul(
                            av[32 * j:32 * (j + 1), :],
                            lhsT=pts[kc][:, t * 512 + ql * P + 32 * j:
                                         t * 512 + ql * P + 32 * (j + 1)],
                            rhs=v_sb[kc][:, h, :],
                            start=(kc == 0), stop=(kc == KB - 1),
                            skip_group_check=True,
                            tile_position=(0, 32 * j))
                rt = spool.tile([P, 1], F32, name="rt")
                nc.vector.reciprocal(out=rt, in_=av[:, D:D + 1])
                a_out = get_a_tiles(qs // 2)[(qs % 2) * 4 + ql][:,
                                                                h * D:(h + 1) * D]
                nc.vector.tensor_scalar_mul(out=a_out, in0=av[:, 0:D],
                                            scalar1=rt)
                if ql == 3 and t == 1:
                    del pts_cur[u]

            def emit_oproj_group(qb, ql, tail=False):
                a_tiles = a_tiles_all[qb]
                att = atpool.tile([P, 2 * P], BF16, name="att")
                for c2 in range(MC):
                    tp = ppp.tile([P, 512], F32, name="pp").bitcast(BF16)[:, 0:P]
                    nc.tensor.transpose(
                        tp, a_tiles[ql][:, c2 * P:(c2 + 1) * P], ident)
                    nc.vector.tensor_copy(out=att[:, c2 * P:(c2 + 1) * P], in_=tp)
                yt = ypool.tile([P, E], BF16, name="yt")
                for nn in range(2):
                    py = ppp.tile([P, 512], F32, name="pp")
                    for c2 in range(MC):
                        nc.tensor.matmul(
                            py, lhsT=att[:, c2 * P:(c2 + 1) * P],
                            rhs=wo_sb[c2][:, nn * 512:(nn + 1) * 512],
                            start=(c2 == 0), stop=(c2 == MC - 1))
                    if tail:    # ACT is idle once the last exp has issued
                        nc.scalar.copy(out=yt[:, nn * 512:(nn + 1) * 512], in_=py)
                    else:
                        nc.vector.tensor_copy(
                            out=yt[:, nn * 512:(nn + 1) * 512], in_=py)
                q0 = qb * 1024 + ql * P
                nc.sync.dma_start(out=y[q0:q0 + P, :], in_=yt)

            def pop_fills(i, kc):
                # kq first (cheap, prerequisite-critical for later units);
                # then V-proj (AV depends on all of it) once xv can be
                # resident; then AV/O-proj (the ptile-recycling path that
                # keeps ACT fed)
                budget = 2 if i >= 1 else 1
                while budget > 0:
                    if kq_fills and (i > 0 or kc >= 2):
                        kq_fills.popleft()()
                    elif v_fills:
                        if i > 0 or kc >= 10:
                            v_fills.popleft()()
                        else:
                            break
                    elif fills:
                        fills.popleft()()
                    else:
                        break
                    budget -= 1

            for kc in range(KB):
                emit_se_chunk(units[0], kc)
                pop_fills(0, kc)
            for i in range(1, len(units) + 1):
                prev = units[i - 1]
                qs, pr = prev
                tail = i == len(units)
                for ql in range(4):
                    for t in range(2):
                        fills.append(
                            lambda u=prev, q=ql, t=t: emit_av_group(u, q, t))
                    if tail and ql >= 1:
                        # final drain only: O-proj block q-1 is ready once
                        # AV pair q-1 is evicted; interleaving pipelines
                        # PE transposes/matmuls against DVE evictions
                        fills.append(
                            lambda b=qs // 2, q=(qs % 2) * 4 + ql - 1:
                            emit_oproj_group(b, q, tail=True))
                if pr == NPAIR - 1 and not tail:
                    for ql in range(4):
                        fills.append(
                            lambda b=qs // 2, q=(qs % 2) * 4 + ql:
                            emit_oproj_group(b, q, tail=False))
                if tail:
                    fills.append(
                        lambda b=qs // 2, q=(qs % 2) * 4 + 3:
                        emit_oproj_group(b, q, tail=True))
                if i < len(units):
                    for kc in range(KB):
                        emit_se_chunk(units[i], kc)
                        pop_fills(i, kc)
                else:
                    for q in (kq_fills, v_fills, fills):
                        while q:
                            q.popleft()()

        for _ in range(reps):
            emit_body()

    _split_drain_waits(nc)
    return nc


_CACHE = {}


def _get_nc():
    if "nc" not in _CACHE:
        _CACHE["nc"] = _build()
    return _CACHE["nc"]


def make_in_maps(query, key, value, Wq, bq, Wk, bk, Wv, bv, Wo, bo, gate):
    query = np.asarray(query, np.float32)
    key = np.asarray(key, np.float32)
    value = np.asarray(value, np.float32)
    Wq = np.asarray(Wq, np.float32)
    Wk = np.asarray(Wk, np.float32)
    Wv = np.asarray(Wv, np.float32)
    Wo = np.asarray(Wo, np.float32)
    bq = np.asarray(bq, np.float32)
    bk = np.asarray(bk, np.float32)
    bv = np.asarray(bv, np.float32)
    gate = np.asarray(gate, np.float32)

    scale_h = (1.0 / (1.0 + np.exp(-gate.astype(np.float64)))
               / math.sqrt(D)).astype(np.float32)

    def pmajor(a, np_, p=P):
        # [np_*p, free] -> [p, np_, free] (partition-major for 1-DMA loads)
        return np.ascontiguousarray(
            a.reshape(np_, p, a.shape[1]).transpose(1, 0, 2))

    xq_b = [pmajor(query[b].T, KC) for b in range(B)]
    xk_b = [pmajor(key[b].T, KC) for b in range(B)]
    xv_b = [pmajor(value[b].T, KC) for b in range(B)]

    in_maps = []
    for core in range(NCORES):
        b, g = divmod(core, GROUPS)
        rows = slice(g * DH, (g + 1) * DH)
        qs = np.repeat(scale_h[g * HG:(g + 1) * HG], D)
        in_maps.append({
            "xqT": xq_b[b], "xkT": xk_b[b], "xvT": xv_b[b],
            "wqT": pmajor(Wq[rows].T, KC),
            "wkT": pmajor(Wk[rows].T, KC),
            "wvT": pmajor(Wv[rows].T, KC),
            "woB": pmajor(Wo[:, rows].T, MC),
            "qscale": np.ascontiguousarray(qs),
            "qbias": np.ascontiguousarray(bq[rows] * qs),
            "kbias": np.ascontiguousarray(bk[rows]),
            "vbias": np.ascontiguousarray(bv[rows]),
        })

    from concourse import mybir as _mb
    bf = _mb.dt.np(_mb.dt.bfloat16)
    for m in in_maps:
        for k in ("xqT", "xkT", "xvT", "wqT", "wkT", "wvT", "woB"):
            m[k] = m[k].astype(bf)
    return in_maps


def assemble(results, query, key, value, Wq, bq, Wk, bk, Wv, bv, Wo, bo, gate):
    bo = np.asarray(bo, np.float32)
    out = np.empty((B, N, E), np.float32)
    for b in range(B):
        acc = results[b * GROUPS]["y"].astype(np.float32)
        for g in range(1, GROUPS):
            acc = acc + results[b * GROUPS + g]["y"].astype(np.float32)
        out[b] = acc + bo
    return out


def kernel(**inputs):
    global LAST_RESULTS
    in_maps = make_in_maps(**inputs)
    res = run_bass_kernel_spmd(_get_nc(), in_maps, list(range(NCORES)),
                               trace=TRACE)
    LAST_RESULTS = res
    return assemble(res.results, **inputs)

